# revision 73
# baseline (speedup 1.0000x reference)
"""Causal GQA self-attention (RMS-normed QK + RoPE + softmax + proj) on 8 trn2 cores.

Sharding: core c = (batch b = c//2, head-group g = c%2).  Each core computes
batch b, q-heads {8g..8g+7}, kv-heads {2g, 2g+1}, and a partial output
projection using Wproj columns for those heads; the host sums the two
partials per batch.

v2: single software-pipelined loop.  Attention row i runs concurrently with
QKV-prep of chunk i+4 and the output projection of chunk i-1; prep/proj
matmuls are spread as cost-budgeted fillers into the attention j-loop
(~520ns/pair, ~2.2us reserved for the row-end dump that covers the
normalize chain while the PSUM accumulators free).  PV is issued with a
2-pair lag behind scores/exp.  Other changes vs v1:
 - all inputs host-prearranged to exact SBUF layout -> one contiguous DMA
   per tensor, priority-ordered across both HWDGE queues (SP + ACT) so
   the first matmul unblocks ~4.5us in.
 - Q^T/K^T produced by XBAR DMA-transpose (no PE transposes, no
   PSUM->SBUF copies, no PSUM transpose tiles).
 - cos/sin tables stored once ([S,64]) and broadcast per head with
   stride-0 APs; q_gain/sqrt(HD) folded into the per-head rsqrt factors
   (host `scl` constant).  qn/kn scaling is a single broadcast
   tensor_tensor each.
 - causal masking of the diagonal block via a broadcast 0/1 mask
   multiply on DVE instead of gpsimd affine_select.
 - PSUM plan (8 banks): scores 2x[128,1024] (4) + PV accumulators
   oa/ob (2) + a single rotating tag shared by q/kv/proj tiles (2).
 - rsqrt via exp-seed + 2 Newton steps (exp-only -> a single activation
   table set, no per-chunk table reloads).
"""

import numpy as np
import ml_dtypes

B, S, D = 4, 2048, 1024
H, KVH, HD = 16, 4, 64
SC = S // 128   # 16 sequence chunks
DC = D // 128   # 8 d_model chunks
QD = 512        # local q dims (8 heads)
EPS = float(np.finfo(np.float32).eps)
ROPE_BASE = 10000.0

_NC_CACHE = {}
_LAST = None  # BassKernelResults of the last run (for test harness introspection)


def _build_bass():
    import concourse.bacc as bacc
    import concourse.mybir as mybir
    import concourse.tile as tile

    dt = mybir.dt
    f32, bf16 = dt.float32, dt.bfloat16
    Alu = mybir.AluOpType
    Act = mybir.ActivationFunctionType
    Ax = mybir.AxisListType

    nc = bacc.Bacc("TRN2", target_bir_lowering=False)

    # all inputs are host-prearranged to the exact SBUF [128, free] layout
    # so every load is one fully-contiguous DMA
    xid = nc.dram_tensor("xi", [128, SC * DC * 128], bf16, kind="ExternalInput")
    wqd = nc.dram_tensor("wq", [128, DC * QD], bf16, kind="ExternalInput")
    wkvd = nc.dram_tensor("wkv", [128, DC * 256], bf16, kind="ExternalInput")
    wpd = nc.dram_tensor("wp", [128, 4 * D], bf16, kind="ExternalInput")
    cqd = nc.dram_tensor("cq", [128, SC * 64], bf16, kind="ExternalInput")
    sqd = nc.dram_tensor("sq", [128, SC * 64], bf16, kind="ExternalInput")
    scld = nc.dram_tensor("scl", [128, 10], f32, kind="ExternalInput")
    tmd = nc.dram_tensor("tm", [128, 128], bf16, kind="ExternalInput")
    yd = nc.dram_tensor("y", [S, D], f32, kind="ExternalOutput")

    with tile.TileContext(nc) as tc:
        with (
            tc.tile_pool(name="per", bufs=1) as per,
        ):
            xt = per.tile([128, SC * DC * 128], bf16, tag="xt")     # [p][i][k][s]
            wq = per.tile([128, DC * QD], bf16, tag="wq")           # [p][k][q]
            wkv = per.tile([128, DC * 256], bf16, tag="wkv")
            wp = per.tile([128, 4 * D], bf16, tag="wp")             # [p][m][d]
            cq = per.tile([128, SC * 64], bf16, tag="cq")           # [p][i][64]
            sq = per.tile([128, SC * 64], bf16, tag="sq")
            scl = per.tile([128, 10], f32, tag="scl")
            tm = per.tile([128, 128], bf16, tag="tm")
            eps_t = per.tile([128, 1], f32, tag="eps")
            half_t = per.tile([128, 1], f32, tag="half")
            QT = per.tile([128, 4 * S], bf16, tag="QT")             # [p][m][s]
            KT = per.tile([128, S], bf16, tag="KT")
            VV = per.tile([128, SC * 130], bf16, tag="VV")
            YT = per.tile([128, 4 * S], bf16, tag="YT")

            nc.vector.memset(eps_t[:], EPS)
            nc.vector.memset(half_t[:], 0.5)
            # ones columns (64, 129) of every VV chunk, in one strided memset
            nc.vector.memset(
                VV[:].rearrange("p (i t c) -> p i t c", t=2, c=65)[:, :, :, 64:65],
                1.0)

            # DMA order matters: prep(0) needs wq+wkv+scl+xt chunks 0-3 first.
            # All transfers are contiguous [128, n] copies (host pre-layout),
            # split across the two HWDGE queues (SP carries weights/cos-sin,
            # ACT carries x) so the first matmul unblocks in ~4.5us.
            xt4 = xt[:].rearrange("p (i k s) -> p i k s", i=SC, k=DC)
            c4 = 4 * DC * 128

            def ld_span(t, td, lo, hi, w):
                nc.sync.dma_start(t[:, lo * w:hi * w], td[:, lo * w:hi * w])

            c1 = DC * 128
            nc.scalar.dma_start(xt[:, 0:c1], xid[:, 0:c1])
            nc.sync.dma_start(wq[:, 0:2 * QD], wqd[:, 0:2 * QD])
            nc.sync.dma_start(wkv[:], wkvd[:, :])
            nc.scalar.dma_start(xt[:, c1:c4], xid[:, c1:c4])
            nc.sync.dma_start(scl[:], scld[:, :])
            nc.sync.dma_start(wq[:, 2 * QD:], wqd[:, 2 * QD:])
            nc.scalar.dma_start(xt[:, c4:2 * c4], xid[:, c4:2 * c4])
            nc.sync.dma_start(cq[:], cqd[:, :])
            nc.sync.dma_start(sq[:], sqd[:, :])
            nc.sync.dma_start(tm[:], tmd[:, :])
            nc.scalar.dma_start(xt[:, 2 * c4:], xid[:, 2 * c4:])
            nc.sync.dma_start(wp[:], wpd[:, :])

            with (
                tc.tile_pool(name="wk", bufs=2) as wk,
                tc.tile_pool(name="ep", bufs=6) as ep,
                tc.tile_pool(name="pmx", bufs=2, space="PSUM") as pmx,
                tc.tile_pool(name="pss", bufs=2, space="PSUM") as pss,
                tc.tile_pool(name="pso", bufs=2, space="PSUM") as pso,
            ):

              def prep_fillers(i, rope_eng=None):
                rope_eng = rope_eng or nc.vector
                """PE matmuls of QKV-prep for chunk i as filler thunks; the
                last thunk chains the non-PE tail (norm, rope, transposes).
                The shared pmx tag rotates q/kv/op/transpose tiles through 2
                PSUM banks; tiles are allocated lazily at issue time so the
                rotation order matches the instruction stream."""
                state = {}

                def q_tile():
                    if "q_ps" not in state:
                        state["q_ps"] = pmx.tile([128, QD], f32, tag="m", name="q_ps")
                    return state["q_ps"]

                def kv_tile():
                    if "kv_ps" not in state:
                        state["kv_ps"] = pmx.tile([128, QD], f32, tag="m", name="kv_ps")
                    return state["kv_ps"]

                fs = []
                for k in range(DC):
                    def qmm(k=k):
                        nc.tensor.matmul(
                            q_tile()[:], xt4[:, i, k], wq[:, k * QD:(k + 1) * QD],
                            start=(k == 0), stop=(k == DC - 1))
                    fs.append((213, qmm))
                for k in range(DC):
                    def kvmm(k=k):
                        nc.tensor.matmul(
                            kv_tile()[:, 0:256], xt4[:, i, k], wkv[:, k * 256:(k + 1) * 256],
                            start=(k == 0), stop=(k == DC - 1))
                    fs.append((107, kvmm))

                def tail_a():
                    q_ps, kv_ps = state["q_ps"], state["kv_ps"]
                    # ---- RMS-norm factors: rs ~ rsqrt(ss/HD + eps) ----
                    q2 = wk.tile([128, QD], f32, tag="q2")
                    nc.scalar.activation(q2[:], q_ps[:], Act.Square)
                    k2 = wk.tile([128, 128], f32, tag="k2")
                    nc.scalar.activation(k2[:], kv_ps[:, 0:128], Act.Square)
                    ss = wk.tile([128, 10], f32, tag="ss")
                    nc.vector.tensor_reduce(
                        ss[:, 0:8], q2[:].rearrange("p (h f) -> p h f", h=8), Ax.X, Alu.add)
                    nc.vector.tensor_reduce(
                        ss[:, 8:10], k2[:].rearrange("p (h f) -> p h f", h=2), Ax.X, Alu.add)
                    lnv = wk.tile([128, 10], f32, tag="lnv")
                    nc.vector.tensor_scalar(lnv[:], ss[:], 1.0 / HD, EPS, Alu.mult, Alu.add)
                    # rsqrt(v): ACT-exp seed exp(0.5-0.5v) ~ v^-0.5 near 1, then
                    # 2 Newton steps y <- y*(1.5 - 0.5*v*y^2) on DVE
                    rs = wk.tile([128, 10], f32, tag="rs")
                    nc.scalar.activation(rs[:], lnv[:], Act.Exp, scale=-0.5, bias=half_t[:])
                    nt = wk.tile([128, 20], f32, tag="nt")
                    for it in range(2):
                        t0 = nt[:, it * 10:(it + 1) * 10]
                        nc.vector.tensor_tensor(t0, rs[:], rs[:], Alu.mult)
                        nc.vector.tensor_tensor(t0, t0, lnv[:], Alu.mult)
                        nc.vector.tensor_scalar(t0, t0, -0.5, 1.5, Alu.mult, Alu.add)
                        nc.vector.tensor_tensor(rs[:], rs[:], t0, Alu.mult)
                    # fold q_gain/sqrt(HD) into the 8 q factors (k cols x1.0)
                    nc.vector.tensor_tensor(rs[:], rs[:], scl[:], Alu.mult)
                    state["rs"] = rs

                def tail_b():
                    q_ps, kv_ps = state["q_ps"], state["kv_ps"]
                    rs = state["rs"]
                    qn = wk.tile([128, QD], bf16, tag="qn")
                    nc.vector.tensor_tensor(
                        qn[:].rearrange("p (h f) -> p h f", h=8),
                        q_ps[:].rearrange("p (h f) -> p h f", h=8),
                        rs[:, 0:8, None].broadcast_to([128, 8, 64]),
                        Alu.mult)
                    kn = wk.tile([128, 128], bf16, tag="kn")
                    nc.vector.tensor_tensor(
                        kn[:].rearrange("p (h f) -> p h f", h=2),
                        kv_ps[:, 0:128].rearrange("p (h f) -> p h f", h=2),
                        rs[:, 8:10, None].broadcast_to([128, 2, 64]),
                        Alu.mult)
                    # ---- V chunk (+ ones cols pre-set) ----
                    vt = VV[:, i * 130:(i + 1) * 130]
                    nc.vector.tensor_copy(vt[:, 0:64], kv_ps[:, 128:192])
                    nc.vector.tensor_copy(vt[:, 65:129], kv_ps[:, 192:256])
                    state["qn"] = qn
                    state["kn"] = kn

                cqi = cq[:, i * 64:(i + 1) * 64]
                sqi = sq[:, i * 64:(i + 1) * 64]

                def tail_c():
                    # ---- RoPE q (DVE, bf16 2x; cos/sin broadcast per head) ----
                    qn = state["qn"]
                    r1 = wk.tile([128, QD], bf16, tag="r1")
                    rope_eng.tensor_tensor(
                        r1[:].rearrange("p (h f) -> p h f", h=8),
                        qn[:].rearrange("p (h f) -> p h f", h=8),
                        cqi[:, None, :].broadcast_to([128, 8, 64]),
                        Alu.mult)
                    r2 = wk.tile([128, QD], bf16, tag="r2")
                    qn3 = qn[:].rearrange("p (h t f) -> p h t f", t=2, f=32)
                    sq3 = sqi.rearrange("p (t f) -> p t f", t=2)
                    r23 = r2[:].rearrange("p (h t f) -> p h t f", t=2, f=32)
                    rope_eng.tensor_tensor(
                        r23[:, :, 0, :], qn3[:, :, 1, :],
                        sq3[:, None, 0, :].broadcast_to([128, 8, 32]), Alu.mult)
                    rope_eng.tensor_tensor(
                        r23[:, :, 1, :], qn3[:, :, 0, :],
                        sq3[:, None, 1, :].broadcast_to([128, 8, 32]), Alu.mult)
                    qr = wk.tile([128, QD], bf16, tag="qr", bufs=4)
                    nc.vector.tensor_tensor(qr[:], r1[:], r2[:], Alu.add)
                    state["qr"] = qr

                def tail_d():
                    # ---- RoPE k ----
                    kn = state["kn"]
                    rk1 = wk.tile([128, 128], bf16, tag="rk1")
                    rope_eng.tensor_tensor(
                        rk1[:].rearrange("p (h f) -> p h f", h=2),
                        kn[:].rearrange("p (h f) -> p h f", h=2),
                        cqi[:, None, :].broadcast_to([128, 2, 64]),
                        Alu.mult)
                    rk2 = wk.tile([128, 128], bf16, tag="rk2")
                    kn3 = kn[:].rearrange("p (h t f) -> p h t f", t=2, f=32)
                    sq3 = sqi.rearrange("p (t f) -> p t f", t=2)
                    rk23 = rk2[:].rearrange("p (h t f) -> p h t f", t=2, f=32)
                    rope_eng.tensor_tensor(
                        rk23[:, :, 0, :], kn3[:, :, 1, :],
                        sq3[:, None, 0, :].broadcast_to([128, 2, 32]), Alu.mult)
                    rope_eng.tensor_tensor(
                        rk23[:, :, 1, :], kn3[:, :, 0, :],
                        sq3[:, None, 1, :].broadcast_to([128, 2, 32]), Alu.mult)
                    kr = wk.tile([128, 128], bf16, tag="kr", bufs=4)
                    nc.vector.tensor_tensor(kr[:], rk1[:], rk2[:], Alu.add)
                    state["kr"] = kr

                def tq():
                    nc.sync.dma_start_transpose(
                        QT[:].rearrange("p (m s) -> p m s", m=4)[:, :, i * 128:(i + 1) * 128],
                        state["qr"][:])

                def tk():
                    nc.sync.dma_start_transpose(KT[:, i * 128:(i + 1) * 128], state["kr"][:])

                return fs, [tail_a, tail_b, tail_c, tail_d, tq, tk], []

              def proj_fillers(ip):
                """Output-projection matmuls for chunk ip; last thunk chains
                the PSUM->SBUF copies (Pool) and the output DMA."""
                osb = wk.tile([128, D], f32, tag="osb")
                ps = [pmx.tile([128, 512], f32, tag="m", name=f"op{dh}")
                      for dh in range(2)]
                fs = []
                for m in range(4):
                    for dh in range(2):
                        def pmm(dh=dh, m=m):
                            nc.tensor.matmul(
                                ps[dh][:],
                                YT[:, m * S + ip * 128:m * S + (ip + 1) * 128],
                                wp[:, m * D + dh * 512:m * D + (dh + 1) * 512],
                                start=(m == 0), stop=(m == 3))
                        fs.append((213, pmm))

                def tail():
                    for dh in range(2):
                        nc.vector.tensor_copy(osb[:, dh * 512:(dh + 1) * 512], ps[dh][:])
                        nc.sync.dma_start(
                            yd[ip * 128:(ip + 1) * 128, dh * 512:(dh + 1) * 512],
                            osb[:, dh * 512:(dh + 1) * 512])

                c, old = fs[-1]

                def last():
                    old()
                    tail()
                fs[-1] = (c, last)
                return fs

              def attn_row(i, fillers, last_proj=None):
                oa = pso.tile([65, QD], f32, tag="o", name="oa")
                ob = pso.tile([65, QD], f32, tag="o", name="ob")
                qt0 = QT[0:64, :].rearrange("p (m s) -> p m s", m=4)[:, :, i * 128:(i + 1) * 128]
                qt1 = QT[64:128, :].rearrange("p (m s) -> p m s", m=4)[:, :, i * 128:(i + 1) * 128]
                ets = {}

                def pv(j, et):
                    nc.tensor.matmul(oa[:], VV[:, j * 130:j * 130 + 65], et[:, 0:512],
                                     start=(j == 0), stop=(j == i))
                    nc.tensor.matmul(ob[:], VV[:, j * 130 + 65:j * 130 + 130], et[:, 512:1024],
                                     start=(j == 0), stop=(j == i))

                for j in range(i + 1):
                    s_ps = pss.tile([128, 1024], f32, tag="s")
                    nc.tensor.matmul(s_ps[:, 0:512], KT[0:64, j * 128:(j + 1) * 128], qt0,
                                     start=True, stop=True)
                    nc.tensor.matmul(s_ps[:, 512:1024], KT[64:128, j * 128:(j + 1) * 128], qt1,
                                     start=True, stop=True)
                    et = ep.tile([128, 1024], bf16, tag="e")
                    nc.scalar.activation(et[:], s_ps[:], Act.Exp)
                    if j == i:
                        et3 = et[:].rearrange("p (b q) -> p b q", q=128)
                        nc.vector.tensor_tensor(
                            et3, et3, tm[:, None, :].broadcast_to([128, 8, 128]),
                            Alu.mult)
                    ets[j] = et
                    # fill the ACT-paced pair gap (~420ns), but keep ~2.8us
                    # of fillers for the row-end dump (covers the normalize
                    # chain while PSUM accumulators free)
                    avail = sum(c for c, _ in fillers) - 2200
                    budget = (min(520, avail // (i + 1 - j))
                              if avail > 0 and i > 5 else 0)
                    while fillers and budget > 0:
                        c, f = fillers.pop(0)
                        f()
                        budget -= c
                    if j >= 2:
                        pv(j - 2, ets.pop(j - 2))
                if i >= 1:
                    pv(i - 1, ets.pop(i - 1))
                pv(i, ets.pop(i))
                fillers[:] = [f for _, f in fillers]
                if last_proj is not None:
                    for f in fillers:
                        f()
                    del fillers[:]

                # ---- normalize and write y^T (issued before the leftover
                # dump so the PSUM accumulators free while PE runs mms) ----
                rcs = []
                for o_ps, nm in ((oa, "rca"), (ob, "rcb")):
                    rc = wk.tile([1, QD], f32, tag="rc", name=nm)
                    nc.vector.reciprocal(rc[:], o_ps[64:65, :])
                    rcs.append(rc)
                rbs = []
                for rc, nm in zip(rcs, ("rba", "rbb")):
                    rb = wk.tile([64, QD], f32, tag="rb", name=nm)
                    nc.gpsimd.partition_broadcast(rb[:], rc[:], channels=64)
                    rbs.append(rb)
                def ymult(g, o_ps, m):
                    out_ap = YT[g * 64:(g + 1) * 64, :].rearrange(
                        "p (m s) -> p m s", m=4)[:, m:m + 1, i * 128:(i + 1) * 128]
                    nc.vector.tensor_tensor(
                        out_ap,
                        o_ps[0:64, :].rearrange("p (m q) -> p m q", m=4)[:, m:m + 1],
                        rbs[g][:].rearrange("p (m q) -> p m q", m=4)[:, m:m + 1],
                        Alu.mult)

                if last_proj is None:
                    for g, o_ps in ((0, oa), (1, ob)):
                        out_ap = YT[g * 64:(g + 1) * 64, :].rearrange(
                            "p (m s) -> p m s", m=4)[:, :, i * 128:(i + 1) * 128]
                        nc.vector.tensor_tensor(
                            out_ap,
                            o_ps[0:64, :].rearrange("p (m q) -> p m q", m=4),
                            rbs[g][:].rearrange("p (m q) -> p m q", m=4),
                            Alu.mult)
                else:
                    # last row: per-m normalize interleaved with the final
                    # projection so the epilogue isn't one serial chain
                    for m in range(4):
                        ymult(0, oa, m)
                        ymult(1, ob, m)
                        _, f0 = last_proj[2 * m]
                        _, f1 = last_proj[2 * m + 1]
                        f0()
                        f1()

                for f in fillers:
                    f()
                del fillers[:]

              # ---- prologue: prep chunks 0-3 (transposes of 2,3 deferred) ----
              trans_q = []
              for c in range(4):
                  mms, tails, trans = prep_fillers(c)
                  # consumption order matched to DMA arrival:
                  # q k=0,1 (wq_a), kv (wkv), q k=2..7 (wq_b)
                  for _, f in mms[0:2] + mms[DC:] + mms[2:DC]:
                      f()
                  for f in tails:
                      f()
                  if c < 2:
                      for _, f in trans:
                          f()
                  else:
                      trans_q.append(trans)
              # ---- main pipelined loop ----
              for i in range(SC):
                  fillers = []
                  if i - 2 >= 0:
                      fillers += proj_fillers(i - 2)
                  if i == SC - 1:
                      fillers += proj_fillers(i - 1)
                  if trans_q:
                      fillers += trans_q.pop(0)
                  post = []
                  if i + 4 < SC:
                      mms, tails, trans = prep_fillers(i + 4)
                      fillers += mms
                      post = tails
                      trans_q.append(trans)
                  lp = proj_fillers(SC - 1) if i == SC - 1 else None
                  attn_row(i, fillers, last_proj=lp)
                  for f in post:
                      f()
              for trans in trans_q:
                  for _, f in trans:
                      f()

    nc.compile()
    return nc


def _get_nc():
    if "nc" not in _NC_CACHE:
        _NC_CACHE["nc"] = _build_bass()
    return _NC_CACHE["nc"]


def _sb(a, rows=128):
    """Host [R*rows, n] -> SBUF layout [rows, R*n]: chunk r lands at free
    offset r*n of each partition."""
    R = a.shape[0] // rows
    return np.ascontiguousarray(
        a.reshape(R, rows, a.shape[1]).transpose(1, 0, 2).reshape(rows, -1))


def _host_consts():
    """Shared (g-independent) host-side constants, in SBUF layout."""
    bf = ml_dtypes.bfloat16
    inv = (1.0 / (ROPE_BASE ** (np.arange(0, HD, 2, dtype=np.float32) / HD))).astype(np.float32)
    th = np.arange(S, dtype=np.float32)[:, None] * inv[None, :]
    cos, sin = np.cos(th).astype(np.float32), np.sin(th).astype(np.float32)
    cfull = np.concatenate([cos, cos], 1)       # [S, 64]
    sfull = np.concatenate([sin, -sin], 1)      # [S, 64] (signs baked)
    cq = _sb(cfull.astype(bf))                  # [128, SC*64]
    sq = _sb(sfull.astype(bf))
    # causal mask for the diagonal block: keep q-col c >= k-partition p
    c = np.arange(128)
    tmask = (c[None, :] >= c[:, None]).astype(bf)       # [128, 128]
    return cq, sq, tmask


def _core_inputs(xb, Wq, Wk, Wv, Wproj, q_gain, g, consts):
    bf = ml_dtypes.bfloat16
    cq, sq, tmask = consts
    qorder = [8 * g + o for o in (0, 4, 1, 5, 2, 6, 3, 7)]

    xT = np.ascontiguousarray(np.asarray(xb, np.float32).T).astype(bf)  # [D, S]
    # xi[p, (i k s)] = xT[k*128+p, i*128+s]
    xi = np.ascontiguousarray(
        xT.reshape(DC, 128, SC, 128).transpose(1, 2, 0, 3).reshape(128, -1))
    Wq_l = np.concatenate([Wq[h * 64:(h + 1) * 64] for h in qorder], 0)  # [512, D]
    wq = _sb(np.ascontiguousarray(Wq_l.T).astype(bf))
    Wk_l = Wk[2 * g * 64:(2 * g + 2) * 64]  # [128, D]
    Wv_l = Wv[2 * g * 64:(2 * g + 2) * 64]
    wkv = _sb(np.ascontiguousarray(np.concatenate([Wk_l, Wv_l], 0).T).astype(bf))
    cols = np.array([(8 * g + m + 4 * half) * 64 + f
                     for m in range(4) for half in range(2) for f in range(64)])
    wp = _sb(np.ascontiguousarray(Wproj[:, cols].T).astype(bf))  # [128, 4*D]

    scale_q = np.asarray(q_gain, np.float32)[qorder] / np.float32(np.sqrt(HD))
    scl = np.ones((128, 10), np.float32)
    scl[:, 0:8] = scale_q[None, :]

    return {"xi": xi, "wq": wq, "wkv": wkv, "wp": wp,
            "cq": cq, "sq": sq, "scl": scl, "tm": tmask}


def kernel(x, Wq, Wk, Wv, Wproj, q_gain):
    global _LAST
    x = np.asarray(x, np.float32)
    Wq = np.asarray(Wq, np.float32)
    Wk = np.asarray(Wk, np.float32)
    Wv = np.asarray(Wv, np.float32)
    Wproj = np.asarray(Wproj, np.float32)
    q_gain = np.asarray(q_gain, np.float32)

    nc = _get_nc()
    consts = _host_consts()
    in_maps = []
    for c in range(8):
        b, g = divmod(c, 2)
        in_maps.append(_core_inputs(x[b], Wq, Wk, Wv, Wproj, q_gain, g, consts))

    from concourse.bass_utils import run_bass_kernel_spmd
    res = run_bass_kernel_spmd(nc, in_maps, core_ids=list(range(8)))
    _LAST = res

    y = np.empty((B, S, D), np.float32)
    for b in range(B):
        y[b] = res.results[2 * b]["y"] + res.results[2 * b + 1]["y"]
    return y


# revision 82
# speedup vs baseline: 1.0040x; 1.0040x over previous
"""Causal GQA self-attention (RMS-normed QK + RoPE + softmax + proj) on 8 trn2 cores.

Sharding: core c = (batch b = c//2, head-group g = c%2).  Each core computes
batch b, q-heads {8g..8g+7}, kv-heads {2g, 2g+1}, and a partial output
projection using Wproj columns for those heads; the host sums the two
partials per batch.

v2: single software-pipelined loop.  Attention row i runs concurrently with
QKV-prep of chunk i+4 and the output projection of chunk i-1; prep/proj
matmuls are spread as cost-budgeted fillers into the attention j-loop
(~440ns/pair, ~1.6us reserved for the row-end dump that covers the
normalize chain while the PSUM accumulators free).  PV is issued with a
2-pair lag behind scores/exp; the output projection is deferred two rows
so it never waits on a just-issued normalize.  Other changes vs v1:
 - all inputs host-prearranged to exact SBUF layout -> one contiguous DMA
   per tensor, priority-ordered across both HWDGE queues (SP + ACT) so
   the first matmul unblocks ~4.5us in.
 - Q^T/K^T produced by XBAR DMA-transpose (no PE transposes, no
   PSUM->SBUF copies, no PSUM transpose tiles).
 - cos/sin tables stored once ([S,64]) and broadcast per head with
   stride-0 APs; q_gain/sqrt(HD) folded into the per-head rsqrt factors
   (host `scl` constant).  qn/kn scaling is a single broadcast
   tensor_tensor each.
 - causal masking of the diagonal block via a broadcast 0/1 mask
   multiply on DVE instead of gpsimd affine_select.
 - PSUM plan (8 banks): scores 2x[128,1024] (4) + PV accumulators
   oa/ob (2) + a single rotating tag shared by q/kv/proj tiles (2).
 - rsqrt via exp-seed + 2 Newton steps (exp-only -> a single activation
   table set, no per-chunk table reloads).
"""

import numpy as np
import ml_dtypes

B, S, D = 4, 2048, 1024
H, KVH, HD = 16, 4, 64
SC = S // 128   # 16 sequence chunks
DC = D // 128   # 8 d_model chunks
QD = 512        # local q dims (8 heads)
EPS = float(np.finfo(np.float32).eps)
ROPE_BASE = 10000.0

_NC_CACHE = {}
_LAST = None  # BassKernelResults of the last run (for test harness introspection)


def _build_bass():
    import concourse.bacc as bacc
    import concourse.mybir as mybir
    import concourse.tile as tile

    dt = mybir.dt
    f32, bf16 = dt.float32, dt.bfloat16
    Alu = mybir.AluOpType
    Act = mybir.ActivationFunctionType
    Ax = mybir.AxisListType

    nc = bacc.Bacc("TRN2", target_bir_lowering=False)

    # all inputs are host-prearranged to the exact SBUF [128, free] layout
    # so every load is one fully-contiguous DMA
    xid = nc.dram_tensor("xi", [128, SC * DC * 128], bf16, kind="ExternalInput")
    wqd = nc.dram_tensor("wq", [128, DC * QD], bf16, kind="ExternalInput")
    wkvd = nc.dram_tensor("wkv", [128, DC * 256], bf16, kind="ExternalInput")
    wpd = nc.dram_tensor("wp", [128, 4 * D], bf16, kind="ExternalInput")
    cqd = nc.dram_tensor("cq", [128, SC * 64], bf16, kind="ExternalInput")
    sqd = nc.dram_tensor("sq", [128, SC * 64], bf16, kind="ExternalInput")
    scld = nc.dram_tensor("scl", [128, 10], f32, kind="ExternalInput")
    tmd = nc.dram_tensor("tm", [128, 128], bf16, kind="ExternalInput")
    yd = nc.dram_tensor("y", [S, D], f32, kind="ExternalOutput")

    with tile.TileContext(nc) as tc:
        with (
            tc.tile_pool(name="per", bufs=1) as per,
        ):
            xt = per.tile([128, SC * DC * 128], bf16, tag="xt")     # [p][i][k][s]
            wq = per.tile([128, DC * QD], bf16, tag="wq")           # [p][k][q]
            wkv = per.tile([128, DC * 256], bf16, tag="wkv")
            wp = per.tile([128, 4 * D], bf16, tag="wp")             # [p][m][d]
            cq = per.tile([128, SC * 64], bf16, tag="cq")           # [p][i][64]
            sq = per.tile([128, SC * 64], bf16, tag="sq")
            scl = per.tile([128, 10], f32, tag="scl")
            tm = per.tile([128, 128], bf16, tag="tm")
            eps_t = per.tile([128, 1], f32, tag="eps")
            half_t = per.tile([128, 1], f32, tag="half")
            QT = per.tile([128, 4 * S], bf16, tag="QT")             # [p][m][s]
            KT = per.tile([128, S], bf16, tag="KT")
            VV = per.tile([128, SC * 130], bf16, tag="VV")
            YT = per.tile([128, 4 * S], bf16, tag="YT")

            nc.vector.memset(eps_t[:], EPS)
            nc.vector.memset(half_t[:], 0.5)
            # ones columns (64, 129) of every VV chunk, in one strided memset
            nc.vector.memset(
                VV[:].rearrange("p (i t c) -> p i t c", t=2, c=65)[:, :, :, 64:65],
                1.0)

            # DMA order matters: prep(0) needs wq+wkv+scl+xt chunks 0-3 first.
            # All transfers are contiguous [128, n] copies (host pre-layout),
            # split across the two HWDGE queues (SP carries weights/cos-sin,
            # ACT carries x) so the first matmul unblocks in ~4.5us.
            xt4 = xt[:].rearrange("p (i k s) -> p i k s", i=SC, k=DC)
            c4 = 4 * DC * 128

            def ld_span(t, td, lo, hi, w):
                nc.sync.dma_start(t[:, lo * w:hi * w], td[:, lo * w:hi * w])

            c1 = DC * 128
            nc.scalar.dma_start(xt[:, 0:c1], xid[:, 0:c1])
            nc.sync.dma_start(wq[:, 0:2 * QD], wqd[:, 0:2 * QD])
            nc.sync.dma_start(wkv[:], wkvd[:, :])
            nc.sync.dma_start(scl[:], scld[:, :])
            nc.sync.dma_start(wq[:, 2 * QD:], wqd[:, 2 * QD:])
            nc.sync.dma_start(cq[:], cqd[:, :])
            nc.sync.dma_start(sq[:], sqd[:, :])
            nc.scalar.dma_start(xt[:, c1:c4], xid[:, c1:c4])
            nc.sync.dma_start(tm[:], tmd[:, :])
            nc.scalar.dma_start(xt[:, c4:2 * c4], xid[:, c4:2 * c4])
            nc.scalar.dma_start(xt[:, 2 * c4:], xid[:, 2 * c4:])
            nc.sync.dma_start(wp[:], wpd[:, :])

            with (
                tc.tile_pool(name="wk", bufs=2) as wk,
                tc.tile_pool(name="ep", bufs=7) as ep,
                tc.tile_pool(name="pmx", bufs=2, space="PSUM") as pmx,
                tc.tile_pool(name="pss", bufs=2, space="PSUM") as pss,
                tc.tile_pool(name="pso", bufs=2, space="PSUM") as pso,
            ):

              def prep_fillers(i, rope_eng=None):
                rope_eng = rope_eng or nc.vector
                """PE matmuls of QKV-prep for chunk i as filler thunks; the
                last thunk chains the non-PE tail (norm, rope, transposes).
                The shared pmx tag rotates q/kv/op/transpose tiles through 2
                PSUM banks; tiles are allocated lazily at issue time so the
                rotation order matches the instruction stream."""
                state = {}

                def q_tile():
                    if "q_ps" not in state:
                        state["q_ps"] = pmx.tile([128, QD], f32, tag="m", name="q_ps")
                    return state["q_ps"]

                def kv_tile():
                    if "kv_ps" not in state:
                        state["kv_ps"] = pmx.tile([128, QD], f32, tag="m", name="kv_ps")
                    return state["kv_ps"]

                fs = []
                for k in range(DC):
                    def qmm(k=k):
                        nc.tensor.matmul(
                            q_tile()[:], xt4[:, i, k], wq[:, k * QD:(k + 1) * QD],
                            start=(k == 0), stop=(k == DC - 1))
                    fs.append((213, qmm))
                for k in range(DC):
                    def kvmm(k=k):
                        nc.tensor.matmul(
                            kv_tile()[:, 0:256], xt4[:, i, k], wkv[:, k * 256:(k + 1) * 256],
                            start=(k == 0), stop=(k == DC - 1))
                    fs.append((107, kvmm))

                def tail_a():
                    q_ps, kv_ps = state["q_ps"], state["kv_ps"]
                    # ---- RMS-norm factors: rs ~ rsqrt(ss/HD + eps) ----
                    q2 = wk.tile([128, QD], f32, tag="q2")
                    nc.scalar.activation(q2[:], q_ps[:], Act.Square)
                    k2 = wk.tile([128, 128], f32, tag="k2")
                    nc.scalar.activation(k2[:], kv_ps[:, 0:128], Act.Square)
                    ss = wk.tile([128, 10], f32, tag="ss")
                    nc.vector.tensor_reduce(
                        ss[:, 0:8], q2[:].rearrange("p (h f) -> p h f", h=8), Ax.X, Alu.add)
                    nc.vector.tensor_reduce(
                        ss[:, 8:10], k2[:].rearrange("p (h f) -> p h f", h=2), Ax.X, Alu.add)
                    lnv = wk.tile([128, 10], f32, tag="lnv")
                    nc.vector.tensor_scalar(lnv[:], ss[:], 1.0 / HD, EPS, Alu.mult, Alu.add)
                    # rsqrt(v): ACT-exp seed exp(0.5-0.5v) ~ v^-0.5 near 1, then
                    # 2 Newton steps y <- y*(1.5 - 0.5*v*y^2) on DVE
                    rs = wk.tile([128, 10], f32, tag="rs")
                    nc.scalar.activation(rs[:], lnv[:], Act.Exp, scale=-0.5, bias=half_t[:])
                    nt = wk.tile([128, 20], f32, tag="nt")
                    for it in range(2):
                        t0 = nt[:, it * 10:(it + 1) * 10]
                        nc.vector.tensor_tensor(t0, rs[:], rs[:], Alu.mult)
                        nc.vector.tensor_tensor(t0, t0, lnv[:], Alu.mult)
                        nc.vector.tensor_scalar(t0, t0, -0.5, 1.5, Alu.mult, Alu.add)
                        nc.vector.tensor_tensor(rs[:], rs[:], t0, Alu.mult)
                    # fold q_gain/sqrt(HD) into the 8 q factors (k cols x1.0)
                    nc.vector.tensor_tensor(rs[:], rs[:], scl[:], Alu.mult)
                    state["rs"] = rs

                def tail_b():
                    q_ps, kv_ps = state["q_ps"], state["kv_ps"]
                    rs = state["rs"]
                    qn = wk.tile([128, QD], bf16, tag="qn")
                    nc.vector.tensor_tensor(
                        qn[:].rearrange("p (h f) -> p h f", h=8),
                        q_ps[:].rearrange("p (h f) -> p h f", h=8),
                        rs[:, 0:8, None].broadcast_to([128, 8, 64]),
                        Alu.mult)
                    kn = wk.tile([128, 128], bf16, tag="kn")
                    nc.vector.tensor_tensor(
                        kn[:].rearrange("p (h f) -> p h f", h=2),
                        kv_ps[:, 0:128].rearrange("p (h f) -> p h f", h=2),
                        rs[:, 8:10, None].broadcast_to([128, 2, 64]),
                        Alu.mult)
                    # ---- V chunk (+ ones cols pre-set) ----
                    vt = VV[:, i * 130:(i + 1) * 130]
                    nc.vector.tensor_copy(vt[:, 0:64], kv_ps[:, 128:192])
                    nc.vector.tensor_copy(vt[:, 65:129], kv_ps[:, 192:256])
                    state["qn"] = qn
                    state["kn"] = kn

                cqi = cq[:, i * 64:(i + 1) * 64]
                sqi = sq[:, i * 64:(i + 1) * 64]

                def tail_c():
                    # ---- RoPE q (DVE, bf16 2x; cos/sin broadcast per head) ----
                    qn = state["qn"]
                    r1 = wk.tile([128, QD], bf16, tag="r1")
                    rope_eng.tensor_tensor(
                        r1[:].rearrange("p (h f) -> p h f", h=8),
                        qn[:].rearrange("p (h f) -> p h f", h=8),
                        cqi[:, None, :].broadcast_to([128, 8, 64]),
                        Alu.mult)
                    r2 = wk.tile([128, QD], bf16, tag="r2")
                    qn3 = qn[:].rearrange("p (h t f) -> p h t f", t=2, f=32)
                    sq3 = sqi.rearrange("p (t f) -> p t f", t=2)
                    r23 = r2[:].rearrange("p (h t f) -> p h t f", t=2, f=32)
                    rope_eng.tensor_tensor(
                        r23[:, :, 0, :], qn3[:, :, 1, :],
                        sq3[:, None, 0, :].broadcast_to([128, 8, 32]), Alu.mult)
                    rope_eng.tensor_tensor(
                        r23[:, :, 1, :], qn3[:, :, 0, :],
                        sq3[:, None, 1, :].broadcast_to([128, 8, 32]), Alu.mult)
                    qr = wk.tile([128, QD], bf16, tag="qr", bufs=4)
                    nc.vector.tensor_tensor(qr[:], r1[:], r2[:], Alu.add)
                    state["qr"] = qr

                def tail_d():
                    # ---- RoPE k ----
                    kn = state["kn"]
                    rk1 = wk.tile([128, 128], bf16, tag="rk1")
                    rope_eng.tensor_tensor(
                        rk1[:].rearrange("p (h f) -> p h f", h=2),
                        kn[:].rearrange("p (h f) -> p h f", h=2),
                        cqi[:, None, :].broadcast_to([128, 2, 64]),
                        Alu.mult)
                    rk2 = wk.tile([128, 128], bf16, tag="rk2")
                    kn3 = kn[:].rearrange("p (h t f) -> p h t f", t=2, f=32)
                    sq3 = sqi.rearrange("p (t f) -> p t f", t=2)
                    rk23 = rk2[:].rearrange("p (h t f) -> p h t f", t=2, f=32)
                    rope_eng.tensor_tensor(
                        rk23[:, :, 0, :], kn3[:, :, 1, :],
                        sq3[:, None, 0, :].broadcast_to([128, 2, 32]), Alu.mult)
                    rope_eng.tensor_tensor(
                        rk23[:, :, 1, :], kn3[:, :, 0, :],
                        sq3[:, None, 1, :].broadcast_to([128, 2, 32]), Alu.mult)
                    kr = wk.tile([128, 128], bf16, tag="kr", bufs=4)
                    nc.vector.tensor_tensor(kr[:], rk1[:], rk2[:], Alu.add)
                    state["kr"] = kr

                def tq():
                    nc.sync.dma_start_transpose(
                        QT[:].rearrange("p (m s) -> p m s", m=4)[:, :, i * 128:(i + 1) * 128],
                        state["qr"][:])

                def tk():
                    nc.sync.dma_start_transpose(KT[:, i * 128:(i + 1) * 128], state["kr"][:])

                return fs, [tail_a, tail_b, tail_c, tail_d, tq, tk], []

              def proj_fillers(ip):
                """Output-projection matmuls for chunk ip; last thunk chains
                the PSUM->SBUF copies (Pool) and the output DMA."""
                osb = wk.tile([128, D], f32, tag="osb")
                ps = [pmx.tile([128, 512], f32, tag="m", name=f"op{dh}")
                      for dh in range(2)]
                fs = []
                for m in range(4):
                    for dh in range(2):
                        def pmm(dh=dh, m=m):
                            nc.tensor.matmul(
                                ps[dh][:],
                                YT[:, m * S + ip * 128:m * S + (ip + 1) * 128],
                                wp[:, m * D + dh * 512:m * D + (dh + 1) * 512],
                                start=(m == 0), stop=(m == 3))
                        fs.append((213, pmm))

                def tail(dh):
                    nc.vector.tensor_copy(osb[:, dh * 512:(dh + 1) * 512], ps[dh][:])
                    nc.sync.dma_start(
                        yd[ip * 128:(ip + 1) * 128, dh * 512:(dh + 1) * 512],
                        osb[:, dh * 512:(dh + 1) * 512])

                for dh in range(2):
                    c, old = fs[6 + dh]

                    def last(old=old, dh=dh):
                        old()
                        tail(dh)
                    fs[6 + dh] = (c, last)
                return fs

              def attn_row(i, fillers, last_proj=None):
                oa = pso.tile([65, QD], f32, tag="o", name="oa")
                ob = pso.tile([65, QD], f32, tag="o", name="ob")
                qt0 = QT[0:64, :].rearrange("p (m s) -> p m s", m=4)[:, :, i * 128:(i + 1) * 128]
                qt1 = QT[64:128, :].rearrange("p (m s) -> p m s", m=4)[:, :, i * 128:(i + 1) * 128]
                ets = {}

                def pv(j, et):
                    nc.tensor.matmul(oa[:], VV[:, j * 130:j * 130 + 65], et[:, 0:512],
                                     start=(j == 0), stop=(j == i))
                    nc.tensor.matmul(ob[:], VV[:, j * 130 + 65:j * 130 + 130], et[:, 512:1024],
                                     start=(j == 0), stop=(j == i))

                for j in range(i + 1):
                    s_ps = pss.tile([128, 1024], f32, tag="s")
                    nc.tensor.matmul(s_ps[:, 0:512], KT[0:64, j * 128:(j + 1) * 128], qt0,
                                     start=True, stop=True)
                    nc.tensor.matmul(s_ps[:, 512:1024], KT[64:128, j * 128:(j + 1) * 128], qt1,
                                     start=True, stop=True)
                    et = ep.tile([128, 1024], bf16, tag="e")
                    nc.scalar.activation(et[:], s_ps[:], Act.Exp)
                    if j == i:
                        et3 = et[:].rearrange("p (b q) -> p b q", q=128)
                        nc.vector.tensor_tensor(
                            et3, et3, tm[:, None, :].broadcast_to([128, 8, 128]),
                            Alu.mult)
                    ets[j] = et
                    # fill the ACT-paced pair gap (~420ns), but keep ~2.8us
                    # of fillers for the row-end dump (covers the normalize
                    # chain while PSUM accumulators free)
                    avail = sum(c for c, _ in fillers) - 1600
                    budget = (min(520, avail // (i + 1 - j))
                              if avail > 0 and i > 5 else 0)
                    while fillers and budget > 0:
                        c, f = fillers.pop(0)
                        f()
                        budget -= c
                    if j >= 3:
                        pv(j - 3, ets.pop(j - 3))
                for jj in (i - 2, i - 1, i):
                    if jj >= 0 and jj in ets:
                        pv(jj, ets.pop(jj))
                fillers[:] = [f for _, f in fillers]
                if last_proj is not None:
                    for f in fillers:
                        f()
                    del fillers[:]

                # ---- normalize and write y^T (issued before the leftover
                # dump so the PSUM accumulators free while PE runs mms) ----
                rcs = []
                for o_ps, nm in ((oa, "rca"), (ob, "rcb")):
                    rc = wk.tile([1, QD], f32, tag="rc", name=nm)
                    nc.vector.reciprocal(rc[:], o_ps[64:65, :])
                    rcs.append(rc)
                rbs = []
                for rc, nm in zip(rcs, ("rba", "rbb")):
                    rb = wk.tile([64, QD], f32, tag="rb", name=nm)
                    nc.gpsimd.partition_broadcast(rb[:], rc[:], channels=64)
                    rbs.append(rb)
                def ymult(g, o_ps, m):
                    out_ap = YT[g * 64:(g + 1) * 64, :].rearrange(
                        "p (m s) -> p m s", m=4)[:, m:m + 1, i * 128:(i + 1) * 128]
                    nc.vector.tensor_tensor(
                        out_ap,
                        o_ps[0:64, :].rearrange("p (m q) -> p m q", m=4)[:, m:m + 1],
                        rbs[g][:].rearrange("p (m q) -> p m q", m=4)[:, m:m + 1],
                        Alu.mult)

                if last_proj is None:
                    for g, o_ps in ((0, oa), (1, ob)):
                        out_ap = YT[g * 64:(g + 1) * 64, :].rearrange(
                            "p (m s) -> p m s", m=4)[:, :, i * 128:(i + 1) * 128]
                        nc.vector.tensor_tensor(
                            out_ap,
                            o_ps[0:64, :].rearrange("p (m q) -> p m q", m=4),
                            rbs[g][:].rearrange("p (m q) -> p m q", m=4),
                            Alu.mult)
                else:
                    # last row: per-m normalize interleaved with the final
                    # projection so the epilogue isn't one serial chain
                    for m in range(4):
                        ymult(0, oa, m)
                        ymult(1, ob, m)
                        _, f0 = last_proj[2 * m]
                        _, f1 = last_proj[2 * m + 1]
                        f0()
                        f1()

                for f in fillers:
                    f()
                del fillers[:]

              # ---- prologue: prep chunks 0-3 (transposes of 2,3 deferred) ----
              trans_q = []
              for c in range(4):
                  mms, tails, trans = prep_fillers(c)
                  # consumption order matched to DMA arrival:
                  # q k=0,1 (wq_a), kv (wkv), q k=2..7 (wq_b)
                  for _, f in mms[0:2] + mms[DC:] + mms[2:DC]:
                      f()
                  for f in tails:
                      f()
                  if c < 2:
                      for _, f in trans:
                          f()
                  else:
                      trans_q.append(trans)
              # ---- main pipelined loop ----
              for i in range(SC):
                  fillers = []
                  if i - 2 >= 0:
                      fillers += proj_fillers(i - 2)
                  if i == SC - 1:
                      fillers += proj_fillers(i - 1)
                  if trans_q:
                      fillers += trans_q.pop(0)
                  post = []
                  if i + 4 < SC:
                      mms, tails, trans = prep_fillers(i + 4)
                      fillers += mms
                      post = tails
                      trans_q.append(trans)
                  lp = proj_fillers(SC - 1) if i == SC - 1 else None
                  attn_row(i, fillers, last_proj=lp)
                  for f in post:
                      f()
              for trans in trans_q:
                  for _, f in trans:
                      f()

    nc.compile()
    return nc


def _get_nc():
    if "nc" not in _NC_CACHE:
        _NC_CACHE["nc"] = _build_bass()
    return _NC_CACHE["nc"]


def _sb(a, rows=128):
    """Host [R*rows, n] -> SBUF layout [rows, R*n]: chunk r lands at free
    offset r*n of each partition."""
    R = a.shape[0] // rows
    return np.ascontiguousarray(
        a.reshape(R, rows, a.shape[1]).transpose(1, 0, 2).reshape(rows, -1))


def _host_consts():
    """Shared (g-independent) host-side constants, in SBUF layout."""
    bf = ml_dtypes.bfloat16
    inv = (1.0 / (ROPE_BASE ** (np.arange(0, HD, 2, dtype=np.float32) / HD))).astype(np.float32)
    th = np.arange(S, dtype=np.float32)[:, None] * inv[None, :]
    cos, sin = np.cos(th).astype(np.float32), np.sin(th).astype(np.float32)
    cfull = np.concatenate([cos, cos], 1)       # [S, 64]
    sfull = np.concatenate([sin, -sin], 1)      # [S, 64] (signs baked)
    cq = _sb(cfull.astype(bf))                  # [128, SC*64]
    sq = _sb(sfull.astype(bf))
    # causal mask for the diagonal block: keep q-col c >= k-partition p
    c = np.arange(128)
    tmask = (c[None, :] >= c[:, None]).astype(bf)       # [128, 128]
    return cq, sq, tmask


def _core_inputs(xb, Wq, Wk, Wv, Wproj, q_gain, g, consts):
    bf = ml_dtypes.bfloat16
    cq, sq, tmask = consts
    qorder = [8 * g + o for o in (0, 4, 1, 5, 2, 6, 3, 7)]

    xT = np.ascontiguousarray(np.asarray(xb, np.float32).T).astype(bf)  # [D, S]
    # xi[p, (i k s)] = xT[k*128+p, i*128+s]
    xi = np.ascontiguousarray(
        xT.reshape(DC, 128, SC, 128).transpose(1, 2, 0, 3).reshape(128, -1))
    Wq_l = np.concatenate([Wq[h * 64:(h + 1) * 64] for h in qorder], 0)  # [512, D]
    wq = _sb(np.ascontiguousarray(Wq_l.T).astype(bf))
    Wk_l = Wk[2 * g * 64:(2 * g + 2) * 64]  # [128, D]
    Wv_l = Wv[2 * g * 64:(2 * g + 2) * 64]
    wkv = _sb(np.ascontiguousarray(np.concatenate([Wk_l, Wv_l], 0).T).astype(bf))
    cols = np.array([(8 * g + m + 4 * half) * 64 + f
                     for m in range(4) for half in range(2) for f in range(64)])
    wp = _sb(np.ascontiguousarray(Wproj[:, cols].T).astype(bf))  # [128, 4*D]

    scale_q = np.asarray(q_gain, np.float32)[qorder] / np.float32(np.sqrt(HD))
    scl = np.ones((128, 10), np.float32)
    scl[:, 0:8] = scale_q[None, :]

    return {"xi": xi, "wq": wq, "wkv": wkv, "wp": wp,
            "cq": cq, "sq": sq, "scl": scl, "tm": tmask}


def kernel(x, Wq, Wk, Wv, Wproj, q_gain):
    global _LAST
    x = np.asarray(x, np.float32)
    Wq = np.asarray(Wq, np.float32)
    Wk = np.asarray(Wk, np.float32)
    Wv = np.asarray(Wv, np.float32)
    Wproj = np.asarray(Wproj, np.float32)
    q_gain = np.asarray(q_gain, np.float32)

    nc = _get_nc()
    consts = _host_consts()
    in_maps = []
    for c in range(8):
        b, g = divmod(c, 2)
        in_maps.append(_core_inputs(x[b], Wq, Wk, Wv, Wproj, q_gain, g, consts))

    from concourse.bass_utils import run_bass_kernel_spmd
    res = run_bass_kernel_spmd(nc, in_maps, core_ids=list(range(8)))
    _LAST = res

    y = np.empty((B, S, D), np.float32)
    for b in range(B):
        y[b] = res.results[2 * b]["y"] + res.results[2 * b + 1]["y"]
    return y
